# revision 19
# baseline (speedup 1.0000x reference)
"""Trainium2 Bass kernel for nn_LocalAttention (T=4096, B=32, H=256, L=512, K=32).

Sharding: data-parallel over batch B across 8 cores (4 batch elements per core).

Per-core dataflow:
  1. coeff phase: kern = lm @ WkP + bk on PE from fp8 Wk (host-prescaled x32),
     col-tiled 4x (m padded to 32), descale-copied to SBUF fp16 by ACT,
     re-laid out to conv-weight layout (partition 32b+k) by one SBUF->SBUF DMA
     per j-group, bk added by DVE.
  2. global phase: glob = lm @ Wg.T + bg on PE (fp32) -> (128h, BC) SBUF.
  3. main stream, hc outer / tt waves / b inner (row groups interleave so the
     4 batches' convs run concurrently): conv psum (128,512) = coef.T @ win4;
     DVE adds enc (fp16 out); ACT tanh per (b,hc,half) on (128,2048) tiles;
     PE scoring ws_pad.T @ tan accumulated into per-b (128,1024) psum at
     rows {0,32,64,96} (col groups = tt%4), mask added via K=1 matmuls.
  4. softmax over T: ACT exp with accum_out row sums, total per b via
     indicator matmul, DVE reciprocal, broadcast matmul, gpsimd normalize,
     DMA out the 4 valid rows per b.
"""

import os
import sys

import numpy as np

if "/opt/trn_rl_repo" not in sys.path:
    sys.path.insert(0, "/opt/trn_rl_repo")

import ml_dtypes

T, B, H, L, K = 4096, 32, 256, 512, 32
NCORES = 8
BC = B // NCORES          # 4 batches per core
HCHUNKS = H // 128        # 2
TTILE = 512
NTT = T // TTILE          # 8 t-tiles
HALF = T // 2             # 2048
WK_SCALE = 32.0

_CACHE = {}


def _build_program(debug_dumps=False):
    import concourse.bacc as bacc
    import concourse.bass as bass
    import concourse.mybir as mybir
    import concourse.tile as tile
    from contextlib import ExitStack

    dt = mybir.dt
    fp32 = dt.float32
    fp16 = dt.float16
    bf16 = dt.bfloat16
    fp8 = dt.float8e4
    ts = bass.ts

    nc = bacc.Bacc(
        "TRN2",
        target_bir_lowering=False,
        debug=False,
        enable_asserts=False,
        num_devices=NCORES,
    )

    # ---------------- dram tensors ----------------
    wkp = nc.dram_tensor("wkp", (128, 4, 16, 512), fp8, kind="ExternalInput").ap()
    lm8 = nc.dram_tensor("lm8", (128, 4, 32), fp8, kind="ExternalInput").ap()
    lmf = nc.dram_tensor("lmf", (128, 4, BC), fp32, kind="ExternalInput").ap()
    wgt = nc.dram_tensor("wgt", (128, 4, H), fp32, kind="ExternalInput").ap()
    bgp = nc.dram_tensor("bgp", (1, H), fp32, kind="ExternalInput").ap()
    bkt = nc.dram_tensor("bkt", (128, HCHUNKS * 128), fp16, kind="ExternalInput").ap()
    wsv = nc.dram_tensor("wsv", (128, HCHUNKS), fp16, kind="ExternalInput").ap()
    win4 = nc.dram_tensor("win4", (128, T), fp16, kind="ExternalInput").ap()
    mskp = nc.dram_tensor("mskp", (BC, T), bf16, kind="ExternalInput").ap()
    enc = nc.dram_tensor("enc", (BC, HCHUNKS, 2, 128, HALF), fp16, kind="ExternalInput").ap()
    att = nc.dram_tensor("att", (BC, 2, 4, 512), fp32, kind="ExternalOutput").ap()
    if debug_dumps:
        d_coef = nc.dram_tensor("d_coef", (128, HCHUNKS * 128), fp16, kind="ExternalOutput").ap()
        d_glob = nc.dram_tensor("d_glob", (128, HCHUNKS, BC), fp32, kind="ExternalOutput").ap()
        d_hid = nc.dram_tensor("d_hid", (128, HALF), fp16, kind="ExternalOutput").ap()
        d_tan = nc.dram_tensor("d_tan", (128, HALF), fp16, kind="ExternalOutput").ap()
        d_exp = nc.dram_tensor("d_exp", (128, 512), fp32, kind="ExternalOutput").ap()
        d_acc = nc.dram_tensor("d_acc", (128, BC, 2), fp32, kind="ExternalOutput").ap()

    TanhF = mybir.ActivationFunctionType.Tanh
    ExpF = mybir.ActivationFunctionType.Exp
    CopyF = mybir.ActivationFunctionType.Copy
    Add = mybir.AluOpType.add

    with tile.TileContext(nc) as tc, ExitStack() as ctx:
        # ---------------- pools ----------------
        small_pool = ctx.enter_context(tc.tile_pool(name="small", bufs=1))
        wk_pool = ctx.enter_context(tc.tile_pool(name="wkpool", bufs=8))
        kst_pool = ctx.enter_context(tc.tile_pool(name="kstage", bufs=2))
        coeff_pool = ctx.enter_context(tc.tile_pool(name="coeffp", bufs=1))
        enc_pool = ctx.enter_context(tc.tile_pool(name="encp", bufs=12))
        hid_pool = ctx.enter_context(tc.tile_pool(name="hidp", bufs=10))
        tan_pool = ctx.enter_context(tc.tile_pool(name="tanp", bufs=6))
        exp_pool = ctx.enter_context(tc.tile_pool(name="expp", bufs=8))
        psum_conv = ctx.enter_context(tc.tile_pool(name="psumc", bufs=4, space="PSUM"))
        psum_sc = ctx.enter_context(tc.tile_pool(name="psums", bufs=2, space="PSUM"))
        psum_misc = ctx.enter_context(tc.tile_pool(name="psumm", bufs=1, space="PSUM"))

        # ---------------- small loads + consts ----------------
        lm8_sb = small_pool.tile([128, 4, 32], fp8)
        nc.sync.dma_start(lm8_sb[:], lm8)
        lmf_sb = small_pool.tile([128, 4, BC], fp32)
        nc.sync.dma_start(lmf_sb[:], lmf)
        wgt_sb = small_pool.tile([128, 4, H], fp32)
        nc.sync.dma_start(wgt_sb[:], wgt)
        bg_sb = small_pool.tile([1, H], fp32)
        nc.sync.dma_start(bg_sb[:], bgp)
        bkt_sb = small_pool.tile([128, HCHUNKS * 128], fp16)
        nc.sync.dma_start(bkt_sb[:], bkt)
        msk_sb = small_pool.tile([128, T], bf16)
        nc.sync.dma_start(
            msk_sb[:].rearrange("(b r) t -> b r t", b=4, r=32)[:, 0:1, :],
            mskp.rearrange("b (o t) -> b o t", o=1),
        )

        ws_pad = small_pool.tile([128, HCHUNKS, 32], fp16)
        nc.vector.memset(ws_pad[:], 0.0)
        nc.sync.dma_start(ws_pad[:, :, 0:1], wsv.rearrange("p (c o) -> p c o", o=1))

        one_pad = small_pool.tile([128, 32], bf16)
        nc.vector.memset(one_pad[:], 0.0)
        for b in range(BC):
            nc.vector.memset(one_pad[32 * b : 32 * b + 1, 0:1], 1.0)
        one_f = small_pool.tile([1, BC], fp32)
        nc.vector.memset(one_f[:], 1.0)
        ind128 = small_pool.tile([128, 1], fp32)
        nc.vector.memset(ind128[:], 0.0)
        for c in range(4):
            nc.vector.memset(ind128[32 * c : 32 * c + 1, :], 1.0)
        ones1x128 = small_pool.tile([1, 128], fp32)
        nc.vector.memset(ones1x128[:], 1.0)

        # ---------------- coeff phase (col-tiled 4x, fp8) ----------------
        # kern[b, j] = (lm @ WkP*32 + 0) / 32 ; j = k*256 + hc*128 + h
        coef_sb = coeff_pool.tile([128, HCHUNKS * 128], fp16)  # partition 32b+k
        kern_tiles = []
        for g in range(4):
            cps = psum_conv.tile([128, 512], fp32, tag="conv", name="cps")
            wk_tiles = {}
            for i in range(4):
                for c in range(4):
                    wk_sb = wk_pool.tile([128, 512], fp8, tag="wk")
                    nc.sync.dma_start(wk_sb[:], wkp[:, i, 4 * g + c, :])
                    wk_tiles[(i, c)] = wk_sb
            for i in range(4):
                for c in range(4):
                    nc.tensor.matmul(
                        cps[32 * c : 32 * c + 32, :],
                        lm8_sb[:, i, :],
                        wk_tiles[(i, c)][:],
                        start=(i == 0),
                        stop=(i == 3),
                        skip_group_check=True,
                        tile_position=(0, 32 * c),
                    )
            kern_g = kst_pool.tile([128, 512], fp16, tag="kst")
            nc.scalar.activation(kern_g[:], cps[:], CopyF, bias=0.0, scale=1.0 / WK_SCALE)
            # relayout: kern_g[32c+b, s*256 + f] -> coef[32b + 8g + 2c + s, f]
            for c in range(4):
                for b in range(BC):
                    p0 = 32 * b + 8 * g + 2 * c
                    nc.sync.dma_start(
                        coef_sb[p0 : p0 + 2, :],
                        kern_g[32 * c + b : 32 * c + b + 1, :].rearrange(
                            "o (s f) -> o s f", s=2
                        ),
                    )
        # bk add (in-place)
        nc.vector.tensor_tensor(coef_sb[:], coef_sb[:], bkt_sb[:], Add)
        if debug_dumps:
            nc.sync.dma_start(d_coef, coef_sb[:])

        # ---------------- global phase (fp32) ----------------
        glob_sb = small_pool.tile([128, HCHUNKS, BC], fp32)
        for hc in range(HCHUNKS):
            gps = psum_misc.tile([128, BC], fp32, tag="misc", name="gps")
            for i in range(4):
                nc.tensor.matmul(
                    gps[:],
                    wgt_sb[:, i, ts(hc, 128)],
                    lmf_sb[:, i, :],
                    start=(i == 0),
                    stop=False,
                )
            nc.tensor.matmul(
                gps[:], bg_sb[:, ts(hc, 128)], one_f[:], start=False, stop=True
            )
            nc.vector.tensor_copy(glob_sb[:, hc, :], gps[:])
        if debug_dumps:
            nc.sync.dma_start(d_glob, glob_sb[:])

        # ---------------- win + enc loads ----------------
        win_sb = small_pool.tile([128, T], fp16)
        nc.sync.dma_start(win_sb[:], win4)
        enc_tiles = {}
        for half in range(2):
            for b in range(BC):
                for hc in range(HCHUNKS):
                    e_sb = enc_pool.tile([128, HALF], fp16, tag="enc")
                    nc.sync.dma_start(e_sb[:], enc[b, hc, half, :, :])
                    enc_tiles[(b, hc, half)] = e_sb

        # ---------------- main stream (hc inner) ----------------
        acc_sb = small_pool.tile([128, BC, 2], fp32)
        exp_tiles = {}
        for half in range(2):
            hid_tiles = {}
            for q in range(4):
                tt = half * 4 + q
                if q == 0:
                    for b in range(BC):
                        for hc in range(HCHUNKS):
                            hid_tiles[(b, hc)] = hid_pool.tile(
                                [128, HALF], fp16, tag="hid",
                                name=f"hid_{half}_{b}_{hc}",
                            )
                cpsums = []
                for b in range(BC):
                    for hc in range(HCHUNKS):
                        cpsum = psum_conv.tile([128, TTILE], fp32, tag="conv")
                        nc.tensor.matmul(
                            cpsum[:],
                            coef_sb[32 * b : 32 * b + 32, ts(hc, 128)],
                            win_sb[32 * b : 32 * b + 32, ts(tt, TTILE)],
                            start=True,
                            stop=True,
                            tile_position=(32 * b, 0),
                        )
                        cpsums.append((b, hc, cpsum))
                for b, hc, cpsum in cpsums:
                    nc.vector.tensor_tensor(
                        hid_tiles[(b, hc)][:, ts(q, TTILE)],
                        cpsum[:],
                        enc_tiles[(b, hc, half)][:, ts(q, TTILE)],
                        Add,
                    )
            for b in range(BC):
                tans = []
                for hc in range(HCHUNKS):
                    tan_sb = tan_pool.tile([128, HALF], fp16, tag="tan",
                                           name=f"tan_{half}_{b}_{hc}")
                    nc.scalar.activation(
                        tan_sb[:],
                        hid_tiles[(b, hc)][:],
                        TanhF,
                        bias=glob_sb[:, hc, b : b + 1],
                        scale=1.0,
                    )
                    tans.append(tan_sb)
                    if debug_dumps and b == 0 and hc == 0 and half == 0:
                        nc.sync.dma_start(d_hid, hid_tiles[(b, hc)][:])
                        nc.sync.dma_start(d_tan, tan_sb[:])
                S_bh = psum_sc.tile([128, 512], fp32, tag="sc",
                                    name=f"S_{half}_{b}")
                for q in range(4):
                    tt = half * 4 + q
                    pos = S_bh[32 * q : 32 * q + 32, :]
                    for hc in range(HCHUNKS):
                        nc.tensor.matmul(
                            pos,
                            ws_pad[:, hc, :],
                            tans[hc][:, ts(q, TTILE)],
                            start=(hc == 0),
                            stop=False,
                            skip_group_check=True,
                            tile_position=(0, 32 * q),
                        )
                    nc.tensor.matmul(
                        pos,
                        one_pad[32 * b : 32 * b + 1, :],
                        msk_sb[32 * b : 32 * b + 1, ts(tt, TTILE)],
                        start=False,
                        stop=True,
                        skip_group_check=True,
                        tile_position=(32 * b, 32 * q),
                    )
                E_bh = exp_pool.tile([128, 512], fp32, tag="exp",
                                     name=f"E_{half}_{b}")
                exp_tiles[(b, half)] = E_bh
                nc.scalar.activation(
                    E_bh[:], S_bh[:], ExpF, bias=0.0, scale=1.0,
                    accum_out=acc_sb[:, b, half : half + 1],
                )
                if debug_dumps and b == 0 and half == 0:
                    nc.sync.dma_start(d_exp, E_bh[:])

        # ---------------- softmax normalize + out ----------------
        tps = psum_misc.tile([1, BC * 2], fp32, tag="misc", name="tps")
        nc.tensor.matmul(
            tps[:], ind128[:], acc_sb[:].rearrange("p b h -> p (b h)"),
            start=True, stop=True,
        )
        tsb = small_pool.tile([1, BC, 2], fp32)
        nc.vector.tensor_copy(tsb[:].rearrange("p b h -> p (b h)"), tps[:])
        tot_sb = small_pool.tile([1, BC], fp32)
        nc.vector.tensor_reduce(
            tot_sb[:], tsb[:], mybir.AxisListType.X, Add
        )
        rec_sb = small_pool.tile([1, BC], fp32)
        nc.vector.reciprocal(rec_sb[:], tot_sb[:])
        bps = psum_misc.tile([128, BC], fp32, tag="misc", name="bps")
        nc.tensor.matmul(bps[:], ones1x128[:], rec_sb[:], start=True, stop=True)
        rec128 = small_pool.tile([128, BC], fp32)
        nc.vector.tensor_copy(rec128[:], bps[:])
        if debug_dumps:
            nc.sync.dma_start(d_acc, acc_sb[:])
        for half in range(2):
            for b in range(BC):
                E_bh = exp_tiles[(b, half)]
                nc.gpsimd.tensor_scalar_mul(E_bh[:], E_bh[:], rec128[:, b : b + 1])
                srcv = E_bh[:].rearrange("(c r) f -> c r f", c=4, r=32)[:, 0:1, :]
                nc.sync.dma_start(
                    att[b, half].rearrange("c (o f) -> c o f", o=1), srcv
                )

    nc.compile()
    return nc


def _get_program():
    if "nc" not in _CACHE:
        _CACHE["nc"] = _build_program()
    return _CACHE["nc"]


def _prep_inputs(encoded_contribution, mask, lm_state, prev_att_weights,
                 Wk, bk, Wg, bg, Ws, bs):
    """Host-side shard + layout prep. Returns list of per-core input dicts."""
    f32 = np.float32
    fp8 = ml_dtypes.float8_e4m3

    enc = np.asarray(encoded_contribution, dtype=f32)
    mask = np.asarray(mask, dtype=f32)
    lm = np.asarray(lm_state, dtype=f32)
    prev = np.asarray(prev_att_weights, dtype=f32)
    Wk = np.asarray(Wk, dtype=f32)
    bk = np.asarray(bk, dtype=f32)
    Wg = np.asarray(Wg, dtype=f32)
    bg = np.asarray(bg, dtype=f32)
    Ws = np.asarray(Ws, dtype=f32)
    bs = np.asarray(bs, dtype=f32)

    # enc: (T, B, H) -> (B, H, T) -> (NCORES, BC, HCHUNKS, 2, 128, HALF)
    enc_t = np.ascontiguousarray(enc.transpose(1, 2, 0).astype(np.float16)).reshape(
        NCORES, BC, HCHUNKS, 128, 2, HALF
    ).transpose(0, 1, 2, 4, 3, 5)
    enc_t = np.ascontiguousarray(enc_t)

    # toeplitz windows: win[b, k, t] = prev_pad[b, k + t]
    prev_pad = np.zeros((B, T + K - 1), dtype=f32)
    prev_pad[:, K - 1 :] = prev.T
    win_full = np.lib.stride_tricks.sliding_window_view(prev_pad, T, axis=1)
    win_full = win_full.astype(np.float16).reshape(NCORES, BC * K, T)  # (8, 128, T)

    # WkP[l, k*256+h] = Wk[h*32+k, l] * 32; dram (128 p, 4 i, 16 j, 512) fp8
    wkp = (
        (Wk * WK_SCALE)
        .reshape(H, K, L)
        .transpose(2, 1, 0)          # (L, K, H)
        .reshape(L, K * H)
        .astype(fp8)
        .reshape(4, 128, 16, 512)
        .transpose(1, 0, 2, 3)
    )
    wkp = np.ascontiguousarray(wkp)

    # bk tiled to conv-coef layout: partition 32b+k, free hc*128+h
    bk_kh = bk.reshape(H, K).T  # (K, H)
    bkt = np.tile(bk_kh.reshape(1, K, HCHUNKS * 128), (BC, 1, 1)).reshape(
        128, HCHUNKS * 128
    ).astype(np.float16)
    bkt = np.ascontiguousarray(bkt)

    # lm chunks: (128, 4, B) fp32; fp8 padded to 32 cols
    lmT = np.ascontiguousarray(lm.T.reshape(4, 128, B).transpose(1, 0, 2))

    wgt = np.ascontiguousarray(Wg.T.reshape(4, 128, H).transpose(1, 0, 2))
    bgp = np.ascontiguousarray(bg.reshape(1, H))
    wsv = np.ascontiguousarray(Ws[0].reshape(HCHUNKS, 128).T).astype(np.float16)

    in_maps = []
    for cidx in range(NCORES):
        lmc = np.ascontiguousarray(lmT[:, :, cidx * BC : (cidx + 1) * BC])
        lm8 = np.zeros((128, 4, 32), dtype=fp8)
        lm8[:, :, :BC] = lmc.astype(fp8)
        m = mask[:, cidx * BC : (cidx + 1) * BC] + bs[0]  # (T, BC)
        mskp = np.ascontiguousarray(m.T).astype(ml_dtypes.bfloat16)
        in_maps.append(
            {
                "wkp": wkp,
                "lm8": lm8,
                "lmf": lmc,
                "wgt": wgt,
                "bgp": bgp,
                "bkt": bkt,
                "wsv": wsv,
                "win4": np.ascontiguousarray(win_full[cidx]),
                "mskp": mskp,
                "enc": np.ascontiguousarray(enc_t[cidx]),
            }
        )
    return in_maps


def _assemble_output(per_core):
    out = np.empty((T, B), dtype=np.float32)
    for cidx in range(NCORES):
        A = np.asarray(per_core[cidx], dtype=np.float32)  # (BC, 2, 4, 512)
        # A[b, half, c, q] = att[(half*4+c)*512 + q, b]
        out[:, cidx * BC : (cidx + 1) * BC] = A.reshape(BC, T).T
    return out


def kernel(**inputs):
    from concourse.bass_utils import run_bass_kernel_spmd

    in_maps = _prep_inputs(**inputs)
    nc = _get_program()
    trace = bool(os.environ.get("BASS_TRACE"))
    res = run_bass_kernel_spmd(nc, in_maps, list(range(NCORES)), trace=trace)
    _CACHE["last_results"] = res
    return _assemble_output([r["att"] for r in res.results])


# revision 21
# speedup vs baseline: 1.7363x; 1.7363x over previous
"""Trainium2 Bass kernel for nn_LocalAttention (T=4096, B=32, H=256, L=512, K=32).

Sharding: data-parallel over batch B across 8 cores (4 batch elements per core).

Per-core dataflow:
  1. coeff phase: kern = lm @ WkP + bk on PE from fp8 Wk (host-prescaled x32),
     col-tiled 4x (m padded to 32), descale-copied to SBUF fp16 by ACT,
     re-laid out to conv-weight layout (partition 32b+k) by one SBUF->SBUF DMA
     per j-group, bk added by DVE.
  2. global phase: glob = lm @ Wg.T + bg on PE (fp32) -> (128h, BC) SBUF.
  3. main stream, hc outer / tt waves / b inner (row groups interleave so the
     4 batches' convs run concurrently): conv psum (128,512) = coef.T @ win4;
     DVE adds enc (fp16 out); ACT tanh per (b,hc,half) on (128,2048) tiles;
     PE scoring ws_pad.T @ tan accumulated into per-b (128,1024) psum at
     rows {0,32,64,96} (col groups = tt%4), mask added via K=1 matmuls.
  4. softmax over T: ACT exp with accum_out row sums, total per b via
     indicator matmul, DVE reciprocal, broadcast matmul, gpsimd normalize,
     DMA out the 4 valid rows per b.
"""

import os
import sys

import numpy as np

if "/opt/trn_rl_repo" not in sys.path:
    sys.path.insert(0, "/opt/trn_rl_repo")

import ml_dtypes

T, B, H, L, K = 4096, 32, 256, 512, 32
NCORES = 8
BC = B // NCORES          # 4 batches per core
HCHUNKS = H // 128        # 2
TTILE = 512
NTT = T // TTILE          # 8 t-tiles
HALF = T // 2             # 2048
WK_SCALE = 32.0

_CACHE = {}


def _build_program(debug_dumps=False):
    import concourse.bacc as bacc
    import concourse.bass as bass
    import concourse.mybir as mybir
    import concourse.tile as tile
    from contextlib import ExitStack

    dt = mybir.dt
    fp32 = dt.float32
    fp16 = dt.float16
    bf16 = dt.bfloat16
    fp8 = dt.float8e4
    ts = bass.ts

    nc = bacc.Bacc(
        "TRN2",
        target_bir_lowering=False,
        debug=False,
        enable_asserts=False,
        num_devices=NCORES,
    )

    # ---------------- dram tensors ----------------
    wkp = nc.dram_tensor("wkp", (128, 4, 16, 512), fp8, kind="ExternalInput").ap()
    lm8 = nc.dram_tensor("lm8", (128, 4, 32), fp8, kind="ExternalInput").ap()
    lmf = nc.dram_tensor("lmf", (128, 4, BC), fp32, kind="ExternalInput").ap()
    wgt = nc.dram_tensor("wgt", (128, 4, H), fp32, kind="ExternalInput").ap()
    bgp = nc.dram_tensor("bgp", (1, H), fp32, kind="ExternalInput").ap()
    bkt = nc.dram_tensor("bkt", (128, HCHUNKS * 128), fp16, kind="ExternalInput").ap()
    wsv = nc.dram_tensor("wsv", (128, HCHUNKS), fp16, kind="ExternalInput").ap()
    win4 = nc.dram_tensor("win4", (128, T), fp16, kind="ExternalInput").ap()
    mskp = nc.dram_tensor("mskp", (128, BC, 1024), bf16, kind="ExternalInput").ap()
    enc = nc.dram_tensor("enc", (BC, HCHUNKS, 2, 128, HALF), fp16, kind="ExternalInput").ap()
    att = nc.dram_tensor("att", (BC, 2, 4, 512), fp32, kind="ExternalOutput").ap()
    if debug_dumps:
        d_coef = nc.dram_tensor("d_coef", (128, HCHUNKS * 128), fp16, kind="ExternalOutput").ap()
        d_glob = nc.dram_tensor("d_glob", (128, HCHUNKS, BC), fp32, kind="ExternalOutput").ap()
        d_hid = nc.dram_tensor("d_hid", (128, HALF), fp16, kind="ExternalOutput").ap()
        d_tan = nc.dram_tensor("d_tan", (128, HALF), fp16, kind="ExternalOutput").ap()
        d_exp = nc.dram_tensor("d_exp", (128, 512), fp32, kind="ExternalOutput").ap()
        d_acc = nc.dram_tensor("d_acc", (128, BC, 2), fp32, kind="ExternalOutput").ap()

    TanhF = mybir.ActivationFunctionType.Tanh
    ExpF = mybir.ActivationFunctionType.Exp
    CopyF = mybir.ActivationFunctionType.Copy
    Add = mybir.AluOpType.add

    with tile.TileContext(nc) as tc, ExitStack() as ctx:
        # ---------------- pools ----------------
        small_pool = ctx.enter_context(tc.tile_pool(name="small", bufs=1))
        wk_pool = ctx.enter_context(tc.tile_pool(name="wkpool", bufs=16))
        kst_pool = ctx.enter_context(tc.tile_pool(name="kstage", bufs=2))
        coeff_pool = ctx.enter_context(tc.tile_pool(name="coeffp", bufs=1))
        enc_pool = ctx.enter_context(tc.tile_pool(name="encp", bufs=16))
        hid_pool = ctx.enter_context(tc.tile_pool(name="hidp", bufs=8))
        tan_pool = ctx.enter_context(tc.tile_pool(name="tanp", bufs=4))
        exp_pool = ctx.enter_context(tc.tile_pool(name="expp", bufs=8))
        psum_conv = ctx.enter_context(tc.tile_pool(name="psumc", bufs=4, space="PSUM"))
        psum_sc = ctx.enter_context(tc.tile_pool(name="psums", bufs=2, space="PSUM"))
        psum_misc = ctx.enter_context(tc.tile_pool(name="psumm", bufs=1, space="PSUM"))

        # ---------------- small loads + consts ----------------
        lm8_sb = small_pool.tile([128, 4, 32], fp8)
        nc.sync.dma_start(lm8_sb[:], lm8)
        lmf_sb = small_pool.tile([128, 4, BC], fp32)
        nc.sync.dma_start(lmf_sb[:], lmf)
        wgt_sb = small_pool.tile([128, 4, H], fp32)
        nc.sync.dma_start(wgt_sb[:], wgt)
        bg_sb = small_pool.tile([1, H], fp32)
        nc.sync.dma_start(bg_sb[:], bgp)
        bkt_sb = small_pool.tile([128, HCHUNKS * 128], fp16)
        nc.sync.dma_start(bkt_sb[:], bkt)
        msk_sb = small_pool.tile([128, BC, 1024], bf16)
        nc.sync.dma_start(msk_sb[:], mskp)

        ws_pad = small_pool.tile([128, HCHUNKS, 32], fp16)
        nc.vector.memset(ws_pad[:], 0.0)
        nc.sync.dma_start(ws_pad[:, :, 0:1], wsv.rearrange("p (c o) -> p c o", o=1))

        one_f = small_pool.tile([1, BC], fp32)
        nc.vector.memset(one_f[:], 1.0)
        ind128 = small_pool.tile([128, 1], fp32)
        nc.vector.memset(ind128[:], 0.0)
        for c in range(4):
            nc.vector.memset(ind128[32 * c : 32 * c + 1, :], 1.0)
        ones1x128 = small_pool.tile([1, 128], fp32)
        nc.vector.memset(ones1x128[:], 1.0)

        # ---------------- coeff phase (col-tiled 4x, fp8) ----------------
        # kern[b, j] = (lm @ WkP*32 + 0) / 32 ; j = k*256 + hc*128 + h
        coef_sb = coeff_pool.tile([128, HCHUNKS * 128], fp16)  # partition 32b+k
        wk_tiles = {}
        for g in range(4):
            for i in range(4):
                wk_sb = wk_pool.tile([128, 2048], fp8, tag="wk",
                                     name=f"wk_{g}_{i}")
                nc.sync.dma_start(
                    wk_sb[:],
                    wkp[:, i, 4 * g : 4 * g + 4, :].rearrange("p c f -> p (c f)"),
                )
                wk_tiles[(g, i)] = wk_sb
        # win + enc loads (issue right behind wk on the SP queue)
        win_sb = small_pool.tile([128, T], fp16)
        nc.sync.dma_start(win_sb[:], win4)
        enc_tiles = {}
        for half in range(2):
            for b in range(BC):
                for hc in range(HCHUNKS):
                    e_sb = enc_pool.tile([128, HALF], fp16, tag="enc",
                                         name=f"enc_{half}_{b}_{hc}")
                    nc.sync.dma_start(e_sb[:], enc[b, hc, half, :, :])
                    enc_tiles[(b, hc, half)] = e_sb
        for g in range(4):
            cps = psum_conv.tile([128, 512], fp32, tag="conv", name="cps")
            for i in range(4):
                for c in range(4):
                    nc.tensor.matmul(
                        cps[32 * c : 32 * c + 32, :],
                        lm8_sb[:, i, :],
                        wk_tiles[(g, i)][:, ts(c, 512)],
                        start=(i == 0),
                        stop=(i == 3),
                        skip_group_check=True,
                        tile_position=(0, 32 * c),
                    )
            kern_g = kst_pool.tile([128, 512], fp16, tag="kst")
            nc.scalar.activation(kern_g[:], cps[:], CopyF, bias=0.0, scale=1.0 / WK_SCALE)
            # relayout: kern_g[32c+b, s*256 + f] -> coef[32b + 8g + 2c + s, f]
            # issued on the ACT hwdge queue so waits don't block SP DMA flow
            for c in range(4):
                for b in range(BC):
                    p0 = 32 * b + 8 * g + 2 * c
                    nc.scalar.dma_start(
                        coef_sb[p0 : p0 + 2, :],
                        kern_g[32 * c + b : 32 * c + b + 1, :].rearrange(
                            "o (s f) -> o s f", s=2
                        ),
                    )
        # bk add (in-place)
        nc.vector.tensor_tensor(coef_sb[:], coef_sb[:], bkt_sb[:], Add)
        if debug_dumps:
            nc.sync.dma_start(d_coef, coef_sb[:])

        # ---------------- global phase (fp32) ----------------
        glob_sb = small_pool.tile([128, HCHUNKS, BC], fp32)
        for hc in range(HCHUNKS):
            gps = psum_misc.tile([128, BC], fp32, tag="misc", name="gps")
            for i in range(4):
                nc.tensor.matmul(
                    gps[:],
                    wgt_sb[:, i, ts(hc, 128)],
                    lmf_sb[:, i, :],
                    start=(i == 0),
                    stop=False,
                )
            nc.tensor.matmul(
                gps[:], bg_sb[:, ts(hc, 128)], one_f[:], start=False, stop=True
            )
            nc.vector.tensor_copy(glob_sb[:, hc, :], gps[:])
        if debug_dumps:
            nc.sync.dma_start(d_glob, glob_sb[:])

        # ---------------- main stream (hc inner) ----------------
        acc_sb = small_pool.tile([128, BC, 2], fp32)
        exp_tiles = {}
        for half in range(2):
            hid_tiles = {}
            for q in range(4):
                tt = half * 4 + q
                if q == 0:
                    for b in range(BC):
                        for hc in range(HCHUNKS):
                            hid_tiles[(b, hc)] = hid_pool.tile(
                                [128, HALF], fp16, tag="hid",
                                name=f"hid_{half}_{b}_{hc}",
                            )
                cpsums = []
                for b in range(BC):
                    for hc in range(HCHUNKS):
                        cpsum = psum_conv.tile([128, TTILE], fp32, tag="conv")
                        nc.tensor.matmul(
                            cpsum[:],
                            coef_sb[32 * b : 32 * b + 32, ts(hc, 128)],
                            win_sb[32 * b : 32 * b + 32, ts(tt, TTILE)],
                            start=True,
                            stop=True,
                            tile_position=(32 * b, 0),
                        )
                        cpsums.append((b, hc, cpsum))
                for b, hc, cpsum in cpsums:
                    nc.vector.tensor_tensor(
                        hid_tiles[(b, hc)][:, ts(q, TTILE)],
                        cpsum[:],
                        enc_tiles[(b, hc, half)][:, ts(q, TTILE)],
                        Add,
                    )
            for b in range(BC):
                tans = []
                for hc in range(HCHUNKS):
                    tan_sb = tan_pool.tile([128, HALF], fp16, tag="tan",
                                           name=f"tan_{half}_{b}_{hc}")
                    nc.scalar.activation(
                        tan_sb[:],
                        hid_tiles[(b, hc)][:],
                        TanhF,
                        bias=glob_sb[:, hc, b : b + 1],
                        scale=1.0,
                    )
                    tans.append(tan_sb)
                    if debug_dumps and b == 0 and hc == 0 and half == 0:
                        nc.sync.dma_start(d_hid, hid_tiles[(b, hc)][:])
                        nc.sync.dma_start(d_tan, tan_sb[:])
                S_bh = psum_sc.tile([128, 512], fp32, tag="sc",
                                    name=f"S_{half}_{b}")
                for q in range(4):
                    tt = half * 4 + q
                    pos = S_bh[32 * q : 32 * q + 32, :]
                    for hc in range(HCHUNKS):
                        nc.tensor.matmul(
                            pos,
                            ws_pad[:, hc, :],
                            tans[hc][:, ts(q, TTILE)],
                            start=(hc == 0),
                            stop=(hc == HCHUNKS - 1),
                            skip_group_check=True,
                            tile_position=(0, 32 * q),
                        )

                nc.vector.tensor_tensor(
                    S_bh[:], S_bh[:], msk_sb[:, b, ts(half, 512)], Add
                )
                E_bh = exp_pool.tile([128, 512], fp32, tag="exp",
                                     name=f"E_{half}_{b}")
                exp_tiles[(b, half)] = E_bh
                nc.scalar.activation(
                    E_bh[:], S_bh[:], ExpF, bias=0.0, scale=1.0,
                    accum_out=acc_sb[:, b, half : half + 1],
                )
                if debug_dumps and b == 0 and half == 0:
                    nc.sync.dma_start(d_exp, E_bh[:])

        # ---------------- softmax normalize + out ----------------
        tps = psum_misc.tile([1, BC * 2], fp32, tag="misc", name="tps")
        nc.tensor.matmul(
            tps[:], ind128[:], acc_sb[:].rearrange("p b h -> p (b h)"),
            start=True, stop=True,
        )
        tsb = small_pool.tile([1, BC, 2], fp32)
        nc.vector.tensor_copy(tsb[:].rearrange("p b h -> p (b h)"), tps[:])
        tot_sb = small_pool.tile([1, BC], fp32)
        nc.vector.tensor_reduce(
            tot_sb[:], tsb[:], mybir.AxisListType.X, Add
        )
        rec_sb = small_pool.tile([1, BC], fp32)
        nc.vector.reciprocal(rec_sb[:], tot_sb[:])
        bps = psum_misc.tile([128, BC], fp32, tag="misc", name="bps")
        nc.tensor.matmul(bps[:], ones1x128[:], rec_sb[:], start=True, stop=True)
        rec128 = small_pool.tile([128, BC], fp32)
        nc.vector.tensor_copy(rec128[:], bps[:])
        if debug_dumps:
            nc.sync.dma_start(d_acc, acc_sb[:])
        for half in range(2):
            for b in range(BC):
                E_bh = exp_tiles[(b, half)]
                nc.vector.tensor_scalar_mul(E_bh[:], E_bh[:], rec128[:, b : b + 1])
                srcv = E_bh[:].rearrange("(c r) f -> c r f", c=4, r=32)[:, 0:1, :]
                nc.sync.dma_start(
                    att[b, half].rearrange("c (o f) -> c o f", o=1), srcv
                )

    nc.compile()
    return nc


def _get_program():
    if "nc" not in _CACHE:
        _CACHE["nc"] = _build_program()
    return _CACHE["nc"]


def _prep_inputs(encoded_contribution, mask, lm_state, prev_att_weights,
                 Wk, bk, Wg, bg, Ws, bs):
    """Host-side shard + layout prep. Returns list of per-core input dicts."""
    f32 = np.float32
    fp8 = ml_dtypes.float8_e4m3

    enc = np.asarray(encoded_contribution, dtype=f32)
    mask = np.asarray(mask, dtype=f32)
    lm = np.asarray(lm_state, dtype=f32)
    prev = np.asarray(prev_att_weights, dtype=f32)
    Wk = np.asarray(Wk, dtype=f32)
    bk = np.asarray(bk, dtype=f32)
    Wg = np.asarray(Wg, dtype=f32)
    bg = np.asarray(bg, dtype=f32)
    Ws = np.asarray(Ws, dtype=f32)
    bs = np.asarray(bs, dtype=f32)

    # enc: (T, B, H) -> (B, H, T) -> (NCORES, BC, HCHUNKS, 2, 128, HALF)
    enc_t = np.ascontiguousarray(enc.transpose(1, 2, 0).astype(np.float16)).reshape(
        NCORES, BC, HCHUNKS, 128, 2, HALF
    ).transpose(0, 1, 2, 4, 3, 5)
    enc_t = np.ascontiguousarray(enc_t)

    # toeplitz windows: win[b, k, t] = prev_pad[b, k + t]
    prev_pad = np.zeros((B, T + K - 1), dtype=f32)
    prev_pad[:, K - 1 :] = prev.T
    win_full = np.lib.stride_tricks.sliding_window_view(prev_pad, T, axis=1)
    win_full = win_full.astype(np.float16).reshape(NCORES, BC * K, T)  # (8, 128, T)

    # WkP[l, k*256+h] = Wk[h*32+k, l] * 32; dram (128 p, 4 i, 16 j, 512) fp8
    wkp = (
        (Wk * WK_SCALE)
        .reshape(H, K, L)
        .transpose(2, 1, 0)          # (L, K, H)
        .reshape(L, K * H)
        .astype(fp8)
        .reshape(4, 128, 16, 512)
        .transpose(1, 0, 2, 3)
    )
    wkp = np.ascontiguousarray(wkp)

    # bk tiled to conv-coef layout: partition 32b+k, free hc*128+h
    bk_kh = bk.reshape(H, K).T  # (K, H)
    bkt = np.tile(bk_kh.reshape(1, K, HCHUNKS * 128), (BC, 1, 1)).reshape(
        128, HCHUNKS * 128
    ).astype(np.float16)
    bkt = np.ascontiguousarray(bkt)

    # lm chunks: (128, 4, B) fp32; fp8 padded to 32 cols
    lmT = np.ascontiguousarray(lm.T.reshape(4, 128, B).transpose(1, 0, 2))

    wgt = np.ascontiguousarray(Wg.T.reshape(4, 128, H).transpose(1, 0, 2))
    bgp = np.ascontiguousarray(bg.reshape(1, H))
    wsv = np.ascontiguousarray(Ws[0].reshape(HCHUNKS, 128).T).astype(np.float16)

    in_maps = []
    for cidx in range(NCORES):
        lmc = np.ascontiguousarray(lmT[:, :, cidx * BC : (cidx + 1) * BC])
        lm8 = np.zeros((128, 4, 32), dtype=fp8)
        lm8[:, :, :BC] = lmc.astype(fp8)
        m = mask[:, cidx * BC : (cidx + 1) * BC] + bs[0]  # (T, BC)
        mp = np.zeros((128, BC, 1024), np.float32)
        for q in range(4):
            for half in range(2):
                tt = half * 4 + q
                mp[32 * q, :, half * 512 : (half + 1) * 512] = m[
                    tt * 512 : (tt + 1) * 512, :
                ].T
        mskp = mp.astype(ml_dtypes.bfloat16)
        in_maps.append(
            {
                "wkp": wkp,
                "lm8": lm8,
                "lmf": lmc,
                "wgt": wgt,
                "bgp": bgp,
                "bkt": bkt,
                "wsv": wsv,
                "win4": np.ascontiguousarray(win_full[cidx]),
                "mskp": mskp,
                "enc": np.ascontiguousarray(enc_t[cidx]),
            }
        )
    return in_maps


def _assemble_output(per_core):
    out = np.empty((T, B), dtype=np.float32)
    for cidx in range(NCORES):
        A = np.asarray(per_core[cidx], dtype=np.float32)  # (BC, 2, 4, 512)
        # A[b, half, c, q] = att[(half*4+c)*512 + q, b]
        out[:, cidx * BC : (cidx + 1) * BC] = A.reshape(BC, T).T
    return out


def kernel(**inputs):
    from concourse.bass_utils import run_bass_kernel_spmd

    in_maps = _prep_inputs(**inputs)
    nc = _get_program()
    trace = bool(os.environ.get("BASS_TRACE"))
    res = run_bass_kernel_spmd(nc, in_maps, list(range(NCORES)), trace=trace)
    _CACHE["last_results"] = res
    return _assemble_output([r["att"] for r in res.results])


# revision 22
# speedup vs baseline: 2.0204x; 1.1636x over previous
"""Trainium2 Bass kernel for nn_LocalAttention (T=4096, B=32, H=256, L=512, K=32).

Sharding: data-parallel over batch B across 8 cores (4 batch elements per core).

Per-core dataflow:
  1. coeff phase: kern = lm @ WkP + bk on PE from fp8 Wk (host-prescaled x32),
     col-tiled 4x (m padded to 32), descale-copied to SBUF fp16 by ACT,
     re-laid out to conv-weight layout (partition 32b+k) by one SBUF->SBUF DMA
     per j-group, bk added by DVE.
  2. global phase: glob = lm @ Wg.T + bg on PE (fp32) -> (128h, BC) SBUF.
  3. main stream, hc outer / tt waves / b inner (row groups interleave so the
     4 batches' convs run concurrently): conv psum (128,512) = coef.T @ win4;
     DVE adds enc (fp16 out); ACT tanh per (b,hc,half) on (128,2048) tiles;
     PE scoring ws_pad.T @ tan accumulated into per-b (128,1024) psum at
     rows {0,32,64,96} (col groups = tt%4), mask added via K=1 matmuls.
  4. softmax over T: ACT exp with accum_out row sums, total per b via
     indicator matmul, DVE reciprocal, broadcast matmul, gpsimd normalize,
     DMA out the 4 valid rows per b.
"""

import os
import sys

import numpy as np

if "/opt/trn_rl_repo" not in sys.path:
    sys.path.insert(0, "/opt/trn_rl_repo")

import ml_dtypes

T, B, H, L, K = 4096, 32, 256, 512, 32
NCORES = 8
BC = B // NCORES          # 4 batches per core
HCHUNKS = H // 128        # 2
TTILE = 512
NTT = T // TTILE          # 8 t-tiles
HALF = T // 2             # 2048
WK_SCALE = 32.0

_CACHE = {}


def _build_program(debug_dumps=False):
    import concourse.bacc as bacc
    import concourse.bass as bass
    import concourse.mybir as mybir
    import concourse.tile as tile
    from contextlib import ExitStack

    dt = mybir.dt
    fp32 = dt.float32
    fp16 = dt.float16
    bf16 = dt.bfloat16
    fp8 = dt.float8e4
    ts = bass.ts

    nc = bacc.Bacc(
        "TRN2",
        target_bir_lowering=False,
        debug=False,
        enable_asserts=False,
        num_devices=NCORES,
    )

    # ---------------- dram tensors ----------------
    wkp = nc.dram_tensor("wkp", (128, 4, 16, 512), fp8, kind="ExternalInput").ap()
    lm8 = nc.dram_tensor("lm8", (128, 4, 32), fp8, kind="ExternalInput").ap()
    lmf = nc.dram_tensor("lmf", (128, 4, BC), fp32, kind="ExternalInput").ap()
    wgt = nc.dram_tensor("wgt", (128, 4, H), fp32, kind="ExternalInput").ap()
    bgp = nc.dram_tensor("bgp", (1, H), fp32, kind="ExternalInput").ap()
    bkt = nc.dram_tensor("bkt", (128, HCHUNKS * 128), fp16, kind="ExternalInput").ap()
    wsv = nc.dram_tensor("wsv", (128, HCHUNKS), fp16, kind="ExternalInput").ap()
    win4 = nc.dram_tensor("win4", (128, T), fp16, kind="ExternalInput").ap()
    mskp = nc.dram_tensor("mskp", (128, BC, 1024), bf16, kind="ExternalInput").ap()
    enc = nc.dram_tensor("enc", (BC, HCHUNKS, 2, 128, HALF), fp16, kind="ExternalInput").ap()
    att = nc.dram_tensor("att", (BC, 2, 4, 512), fp32, kind="ExternalOutput").ap()
    if debug_dumps:
        d_coef = nc.dram_tensor("d_coef", (128, HCHUNKS * 128), fp16, kind="ExternalOutput").ap()
        d_glob = nc.dram_tensor("d_glob", (128, HCHUNKS, BC), fp32, kind="ExternalOutput").ap()
        d_hid = nc.dram_tensor("d_hid", (128, HALF), fp16, kind="ExternalOutput").ap()
        d_tan = nc.dram_tensor("d_tan", (128, HALF), fp16, kind="ExternalOutput").ap()
        d_exp = nc.dram_tensor("d_exp", (128, 512), fp32, kind="ExternalOutput").ap()
        d_acc = nc.dram_tensor("d_acc", (128, BC, 2), fp32, kind="ExternalOutput").ap()

    TanhF = mybir.ActivationFunctionType.Tanh
    ExpF = mybir.ActivationFunctionType.Exp
    CopyF = mybir.ActivationFunctionType.Copy
    Add = mybir.AluOpType.add

    with tile.TileContext(nc) as tc, ExitStack() as ctx:
        # ---------------- pools ----------------
        small_pool = ctx.enter_context(tc.tile_pool(name="small", bufs=1))
        wk_pool = ctx.enter_context(tc.tile_pool(name="wkpool", bufs=4))
        kst_pool = ctx.enter_context(tc.tile_pool(name="kstage", bufs=2))
        coeff_pool = ctx.enter_context(tc.tile_pool(name="coeffp", bufs=1))
        enc_pool = ctx.enter_context(tc.tile_pool(name="encp", bufs=2))
        hid_pool = ctx.enter_context(tc.tile_pool(name="hidp", bufs=9))
        tan_pool = ctx.enter_context(tc.tile_pool(name="tanp", bufs=4))
        exp_pool = ctx.enter_context(tc.tile_pool(name="expp", bufs=8))
        psum_conv = ctx.enter_context(tc.tile_pool(name="psumc", bufs=4, space="PSUM"))
        psum_sc = ctx.enter_context(tc.tile_pool(name="psums", bufs=2, space="PSUM"))
        psum_misc = ctx.enter_context(tc.tile_pool(name="psumm", bufs=1, space="PSUM"))
        dram_pool = ctx.enter_context(tc.tile_pool(name="dramp", bufs=1, space="DRAM"))

        # ---------------- ACT-ring DMAs: win + enc (issued up front) -----
        win_sb = small_pool.tile([128, T], fp16)
        nc.scalar.dma_start(win_sb[:], win4)
        enc_half = {}
        for half in range(2):
            e_sb = enc_pool.tile([128, BC, HCHUNKS, HALF], fp16, tag="enc",
                                 name=f"enc_{half}")
            enc_half[half] = e_sb
            for q in range(4):
                nc.scalar.dma_start(
                    e_sb[:, :, :, ts(q, 512)],
                    enc[:, :, half, :, ts(q, 512)].rearrange(
                        "b hc p f -> p b hc f"
                    ),
                )

        # ---------------- SP-ring DMAs: lm8 + wk + smalls ----------------
        lm8_sb = small_pool.tile([128, 4, 32], fp8)
        nc.sync.dma_start(lm8_sb[:], lm8)
        wk_tiles = []
        for g in range(4):
            wk_sb = wk_pool.tile([128, 4, 4, 512], fp8, tag="wk",
                                 name=f"wk_{g}")
            nc.sync.dma_start(wk_sb[:], wkp[:, :, 4 * g : 4 * g + 4, :])
            wk_tiles.append(wk_sb)
        lmf_sb = small_pool.tile([128, 4, BC], fp32)
        nc.sync.dma_start(lmf_sb[:], lmf)
        wgt_sb = small_pool.tile([128, 4, H], fp32)
        nc.sync.dma_start(wgt_sb[:], wgt)
        bg_sb = small_pool.tile([1, H], fp32)
        nc.sync.dma_start(bg_sb[:], bgp)
        bkt_sb = small_pool.tile([128, HCHUNKS * 128], fp16)
        nc.sync.dma_start(bkt_sb[:], bkt)
        msk_sb = small_pool.tile([128, BC, 1024], bf16)
        nc.sync.dma_start(msk_sb[:], mskp)
        ws_pad = small_pool.tile([128, HCHUNKS, 32], fp16)
        nc.vector.memset(ws_pad[:], 0.0)
        nc.sync.dma_start(ws_pad[:, :, 0:1], wsv.rearrange("p (c o) -> p c o", o=1))

        one_f = small_pool.tile([1, BC], fp32)
        nc.vector.memset(one_f[:], 1.0)
        ind128 = small_pool.tile([128, 1], fp32)
        nc.vector.memset(ind128[:], 0.0)
        for c in range(4):
            nc.vector.memset(ind128[32 * c : 32 * c + 1, :], 1.0)
        ones1x128 = small_pool.tile([1, 128], fp32)
        nc.vector.memset(ones1x128[:], 1.0)

        # ---------------- coeff phase (col-tiled 4x, fp8) ----------------
        # kern[b, j] = (lm @ WkP*32) / 32 ; j = k*256 + hc*128 + h
        coef_sb = coeff_pool.tile([128, HCHUNKS * 128], fp16)  # partition 32b+k
        scr = dram_pool.tile([4, 128, 512], fp16)
        for g in range(4):
            cps = psum_conv.tile([128, 512], fp32, tag="conv", name="cps")
            for i in range(4):
                for c in range(4):
                    nc.tensor.matmul(
                        cps[32 * c : 32 * c + 32, :],
                        lm8_sb[:, i, :],
                        wk_tiles[g][:, i, c, :],
                        start=(i == 0),
                        stop=(i == 3),
                        skip_group_check=True,
                        tile_position=(0, 32 * c),
                    )
            kern_g = kst_pool.tile([128, 512], fp16, tag="kst")
            nc.scalar.activation(kern_g[:], cps[:], CopyF, bias=0.0, scale=1.0 / WK_SCALE)
            nc.sync.dma_start(scr[g], kern_g[:])
        # bounce back: coef[32b + 8g + 2c + s, f] = scr[g, 32c+b, s*256+f]
        srcv = scr[:].rearrange("g (c b) (s f) -> b g c s f", c=4, b=32, s=2)
        for b in range(BC):
            nc.sync.dma_start(coef_sb[32 * b : 32 * b + 32, :], srcv[b])
        # bk add (in-place)
        nc.vector.tensor_tensor(coef_sb[:], coef_sb[:], bkt_sb[:], Add)
        if debug_dumps:
            nc.sync.dma_start(d_coef, coef_sb[:])

        # ---------------- global phase (fp32) ----------------
        glob_sb = small_pool.tile([128, HCHUNKS, BC], fp32)
        for hc in range(HCHUNKS):
            gps = psum_misc.tile([128, BC], fp32, tag="misc", name="gps")
            for i in range(4):
                nc.tensor.matmul(
                    gps[:],
                    wgt_sb[:, i, ts(hc, 128)],
                    lmf_sb[:, i, :],
                    start=(i == 0),
                    stop=False,
                )
            nc.tensor.matmul(
                gps[:], bg_sb[:, ts(hc, 128)], one_f[:], start=False, stop=True
            )
            nc.vector.tensor_copy(glob_sb[:, hc, :], gps[:])
        if debug_dumps:
            nc.sync.dma_start(d_glob, glob_sb[:])

        # ---------------- main stream ----------------
        acc_sb = small_pool.tile([128, BC, 2], fp32)
        exp_tiles = {}
        for half in range(2):
            hid_tiles = {}
            for q in range(4):
                tt = half * 4 + q
                if q == 0:
                    for b in range(BC):
                        for hc in range(HCHUNKS):
                            hid_tiles[(b, hc)] = hid_pool.tile(
                                [128, HALF], fp16, tag="hid",
                                name=f"hid_{half}_{b}_{hc}",
                            )
                cpsums = []
                for hc in range(HCHUNKS):
                    for b in range(BC):
                        cpsum = psum_conv.tile([128, TTILE], fp32, tag="conv")
                        nc.tensor.matmul(
                            cpsum[:],
                            coef_sb[32 * b : 32 * b + 32, ts(hc, 128)],
                            win_sb[32 * b : 32 * b + 32, ts(tt, TTILE)],
                            start=True,
                            stop=True,
                            tile_position=(32 * b, 0),
                        )
                        cpsums.append((b, hc, cpsum))
                for b, hc, cpsum in cpsums:
                    nc.vector.tensor_tensor(
                        hid_tiles[(b, hc)][:, ts(q, TTILE)],
                        cpsum[:],
                        enc_half[half][:, b, hc, ts(q, TTILE)],
                        Add,
                    )
            for b in range(BC):
                tans = []
                for hc in range(HCHUNKS):
                    tan_sb = tan_pool.tile([128, HALF], fp16, tag="tan",
                                           name=f"tan_{half}_{b}_{hc}")
                    nc.scalar.activation(
                        tan_sb[:],
                        hid_tiles[(b, hc)][:],
                        TanhF,
                        bias=glob_sb[:, hc, b : b + 1],
                        scale=1.0,
                    )
                    tans.append(tan_sb)
                    if debug_dumps and b == 0 and hc == 0 and half == 0:
                        nc.sync.dma_start(d_hid, hid_tiles[(b, hc)][:])
                        nc.sync.dma_start(d_tan, tan_sb[:])
                S_bh = psum_sc.tile([128, 512], fp32, tag="sc",
                                    name=f"S_{half}_{b}")
                for hc in range(HCHUNKS):
                    for q in range(4):
                        nc.tensor.matmul(
                            S_bh[32 * q : 32 * q + 32, :],
                            ws_pad[:, hc, :],
                            tans[hc][:, ts(q, TTILE)],
                            start=(hc == 0),
                            stop=(hc == HCHUNKS - 1),
                            skip_group_check=True,
                            tile_position=(0, 32 * q),
                        )
                nc.vector.tensor_tensor(
                    S_bh[:], S_bh[:], msk_sb[:, b, ts(half, 512)], Add
                )
                E_bh = exp_pool.tile([128, 512], fp32, tag="exp",
                                     name=f"E_{half}_{b}")
                exp_tiles[(b, half)] = E_bh
                nc.scalar.activation(
                    E_bh[:], S_bh[:], ExpF, bias=0.0, scale=1.0,
                    accum_out=acc_sb[:, b, half : half + 1],
                )
                if debug_dumps and b == 0 and half == 0:
                    nc.sync.dma_start(d_exp, E_bh[:])

        # ---------------- softmax normalize + out ----------------
        tps = psum_misc.tile([1, BC * 2], fp32, tag="misc", name="tps")
        nc.tensor.matmul(
            tps[:], ind128[:], acc_sb[:].rearrange("p b h -> p (b h)"),
            start=True, stop=True,
        )
        tsb = small_pool.tile([1, BC, 2], fp32)
        nc.vector.tensor_copy(tsb[:].rearrange("p b h -> p (b h)"), tps[:])
        tot_sb = small_pool.tile([1, BC], fp32)
        nc.vector.tensor_reduce(
            tot_sb[:], tsb[:], mybir.AxisListType.X, Add
        )
        rec_sb = small_pool.tile([1, BC], fp32)
        nc.vector.reciprocal(rec_sb[:], tot_sb[:])
        bps = psum_misc.tile([128, BC], fp32, tag="misc", name="bps")
        nc.tensor.matmul(bps[:], ones1x128[:], rec_sb[:], start=True, stop=True)
        rec128 = small_pool.tile([128, BC], fp32)
        nc.vector.tensor_copy(rec128[:], bps[:])
        if debug_dumps:
            nc.sync.dma_start(d_acc, acc_sb[:])
        for half in range(2):
            for b in range(BC):
                E_bh = exp_tiles[(b, half)]
                nc.vector.tensor_scalar_mul(E_bh[:], E_bh[:], rec128[:, b : b + 1])
                srcv2 = E_bh[:].rearrange("(c r) f -> c r f", c=4, r=32)[:, 0:1, :]
                eng = nc.sync if (half * BC + b) % 2 == 0 else nc.scalar
                eng.dma_start(
                    att[b, half].rearrange("c (o f) -> c o f", o=1), srcv2
                )

    nc.compile()
    return nc


def _get_program():
    if "nc" not in _CACHE:
        _CACHE["nc"] = _build_program()
    return _CACHE["nc"]


def _prep_inputs(encoded_contribution, mask, lm_state, prev_att_weights,
                 Wk, bk, Wg, bg, Ws, bs):
    """Host-side shard + layout prep. Returns list of per-core input dicts."""
    f32 = np.float32
    fp8 = ml_dtypes.float8_e4m3

    enc = np.asarray(encoded_contribution, dtype=f32)
    mask = np.asarray(mask, dtype=f32)
    lm = np.asarray(lm_state, dtype=f32)
    prev = np.asarray(prev_att_weights, dtype=f32)
    Wk = np.asarray(Wk, dtype=f32)
    bk = np.asarray(bk, dtype=f32)
    Wg = np.asarray(Wg, dtype=f32)
    bg = np.asarray(bg, dtype=f32)
    Ws = np.asarray(Ws, dtype=f32)
    bs = np.asarray(bs, dtype=f32)

    # enc: (T, B, H) -> (B, H, T) -> (NCORES, BC, HCHUNKS, 2, 128, HALF)
    enc_t = np.ascontiguousarray(enc.transpose(1, 2, 0).astype(np.float16)).reshape(
        NCORES, BC, HCHUNKS, 128, 2, HALF
    ).transpose(0, 1, 2, 4, 3, 5)
    enc_t = np.ascontiguousarray(enc_t)

    # toeplitz windows: win[b, k, t] = prev_pad[b, k + t]
    prev_pad = np.zeros((B, T + K - 1), dtype=f32)
    prev_pad[:, K - 1 :] = prev.T
    win_full = np.lib.stride_tricks.sliding_window_view(prev_pad, T, axis=1)
    win_full = win_full.astype(np.float16).reshape(NCORES, BC * K, T)  # (8, 128, T)

    # WkP[l, k*256+h] = Wk[h*32+k, l] * 32; dram (128 p, 4 i, 16 j, 512) fp8
    wkp = (
        (Wk * WK_SCALE)
        .reshape(H, K, L)
        .transpose(2, 1, 0)          # (L, K, H)
        .reshape(L, K * H)
        .astype(fp8)
        .reshape(4, 128, 16, 512)
        .transpose(1, 0, 2, 3)
    )
    wkp = np.ascontiguousarray(wkp)

    # bk tiled to conv-coef layout: partition 32b+k, free hc*128+h
    bk_kh = bk.reshape(H, K).T  # (K, H)
    bkt = np.tile(bk_kh.reshape(1, K, HCHUNKS * 128), (BC, 1, 1)).reshape(
        128, HCHUNKS * 128
    ).astype(np.float16)
    bkt = np.ascontiguousarray(bkt)

    # lm chunks: (128, 4, B) fp32; fp8 padded to 32 cols
    lmT = np.ascontiguousarray(lm.T.reshape(4, 128, B).transpose(1, 0, 2))

    wgt = np.ascontiguousarray(Wg.T.reshape(4, 128, H).transpose(1, 0, 2))
    bgp = np.ascontiguousarray(bg.reshape(1, H))
    wsv = np.ascontiguousarray(Ws[0].reshape(HCHUNKS, 128).T).astype(np.float16)

    in_maps = []
    for cidx in range(NCORES):
        lmc = np.ascontiguousarray(lmT[:, :, cidx * BC : (cidx + 1) * BC])
        lm8 = np.zeros((128, 4, 32), dtype=fp8)
        lm8[:, :, :BC] = lmc.astype(fp8)
        m = mask[:, cidx * BC : (cidx + 1) * BC] + bs[0]  # (T, BC)
        mp = np.zeros((128, BC, 1024), np.float32)
        for q in range(4):
            for half in range(2):
                tt = half * 4 + q
                mp[32 * q, :, half * 512 : (half + 1) * 512] = m[
                    tt * 512 : (tt + 1) * 512, :
                ].T
        mskp = mp.astype(ml_dtypes.bfloat16)
        in_maps.append(
            {
                "wkp": wkp,
                "lm8": lm8,
                "lmf": lmc,
                "wgt": wgt,
                "bgp": bgp,
                "bkt": bkt,
                "wsv": wsv,
                "win4": np.ascontiguousarray(win_full[cidx]),
                "mskp": mskp,
                "enc": np.ascontiguousarray(enc_t[cidx]),
            }
        )
    return in_maps


def _assemble_output(per_core):
    out = np.empty((T, B), dtype=np.float32)
    for cidx in range(NCORES):
        A = np.asarray(per_core[cidx], dtype=np.float32)  # (BC, 2, 4, 512)
        # A[b, half, c, q] = att[(half*4+c)*512 + q, b]
        out[:, cidx * BC : (cidx + 1) * BC] = A.reshape(BC, T).T
    return out


def kernel(**inputs):
    from concourse.bass_utils import run_bass_kernel_spmd

    in_maps = _prep_inputs(**inputs)
    nc = _get_program()
    trace = bool(os.environ.get("BASS_TRACE"))
    res = run_bass_kernel_spmd(nc, in_maps, list(range(NCORES)), trace=trace)
    _CACHE["last_results"] = res
    return _assemble_output([r["att"] for r in res.results])
